# revision 9
# baseline (speedup 1.0000x reference)
"""AttnLSTMEmbedding kernel for 8 Trainium2 NeuronCores (Bass/Tile).

Strategy (hardcoded for n_test=512, n_support=2048, n_feat=2048, 10 steps):
  - Tensor-parallel over the 4*n_feat gate dim: core k owns 1024 gate columns
    (256 per gate), holding [W1; W2; U] as a [6144, 1024] bf16 slice resident
    in SBUF.  All activations are kept feature-major ("transposed", [feat, test])
    so no on-device transposes are ever needed.
  - Attention is sharded over the support dim: core k owns support rows
    [256k, 256k+256) in both layouts (xp^T for the logit matmul, xp for the
    value matmul with a fused ones-column computing the softmax denominator).
    Per-step AllReduce of the [2049, 512] bf16 numerator, hidden under the
    W1/U gate matmuls.
  - Per-step AllGather of the (o, h) feature slices rebuilds the full
    activations on every core.
  - q is never materialized: z = xq@W1 + r@W2 + h@U + (b - x@W1), with the
    constant C = b - x@W1 precomputed on the host in float64.
  - The global softmax scale 1/sqrt(sum(xq^2)*sum(xp^2)) is computed on-device
    (DVE square-accumulate + ones-matmul partition fold) and fused into the
    Exp activation's scale operand.
"""

import numpy as np
import ml_dtypes

import concourse.bass as bass
import concourse.bacc as bacc
import concourse.tile as tile
import concourse.mybir as mybir
import bass_rust
from concourse.bass_utils import run_bass_kernel_spmd

BF16 = mybir.dt.bfloat16
F32 = mybir.dt.float32
AX = bass_rust.AxisListType.X
OP = mybir.AluOpType
AF = mybir.ActivationFunctionType
nbf = ml_dtypes.bfloat16

NCORES = 8
T = 512          # n_test
F = 2048         # n_feat
S = 2048         # n_support
STEPS = 10
FS = F // NCORES          # 256 features per core
SS = S // NCORES          # 256 support rows per core
G = 4 * FS                # 1024 gate columns per core
KT = 128
NT16 = F // KT            # 16 k-tiles for a [2048, 512] operand
NKW = (3 * F) // KT       # 48 k-tiles of the fused weight matrix
NM = G // KT              # 8 M-tiles of z
NMA = S + 1               # 2049 rows of the augmented numerator


def _build(s1_scale: float, sxp: float, trace_steps: bool = False):
    """Build the SPMD program. s1_scale = 1/denom for step 1 (host-baked),
    sxp = sum(xp^2) baked into the on-device denom for steps 2+."""
    nc = bacc.Bacc("TRN2", target_bir_lowering=False, debug=False,
                   num_devices=NCORES)

    wt_d = nc.dram_tensor("wt", [3 * F, G], BF16, kind="ExternalInput")
    ctp_d = nc.dram_tensor("ctp", [G, T], F32, kind="ExternalInput")
    xpt_d = nc.dram_tensor("xpt", [F, SS], BF16, kind="ExternalInput")
    xp1_d = nc.dram_tensor("xp1", [SS, NMA], BF16, kind="ExternalInput")
    xt_d = nc.dram_tensor("xt", [F, T], BF16, kind="ExternalInput")
    o_out_d = nc.dram_tensor("o_out", [FS, T], F32, kind="ExternalOutput")

    rg = [list(range(NCORES))]

    with tile.TileContext(nc) as tc:
        with (
            tc.tile_pool(name="res", bufs=1) as res,
            tc.tile_pool(name="stage", bufs=4) as stage,
            tc.tile_pool(name="psum", bufs=8, space="PSUM") as psum,
            tc.tile_pool(name="dram", bufs=2, space="DRAM") as dram,
        ):
            # ---- resident SBUF tensors ----
            wt_s = res.tile([KT, NKW * G], BF16, tag="wt")        # 96 KB/p
            xpt_s = res.tile([KT, NT16 * SS], BF16, tag="xpt")    # 8 KB/p
            xp1_s = res.tile([KT, 2 * NMA], BF16, tag="xp1")      # 8 KB/p
            xt_s = res.tile([KT, NT16 * T], BF16, tag="xt")       # 16 KB/p
            xq_s = res.tile([KT, NT16 * T], BF16, tag="xq")       # 16 KB/p
            ht_s = res.tile([KT, NT16 * T], BF16, tag="ht")       # 16 KB/p
            rt_s = res.tile([KT, NT16 * T], BF16, tag="rt")       # 16 KB/p
            expe_s = res.tile([KT, 2 * T], BF16, tag="expe")
            c_s = res.tile([KT, 2 * T], F32, tag="c")             # 4 KB/p
            i_bf = res.tile([KT, 2 * T], BF16, tag="i")
            f_bf = res.tile([KT, 2 * T], BF16, tag="f")
            tcand_bf = res.tile([KT, 2 * T], BF16, tag="tcand")
            tanhc_bf = res.tile([KT, 2 * T], BF16, tag="tanhc")
            tmp_f = res.tile([KT, T], F32, tag="tmpf")            # 2 KB/p
            ag_stage = res.tile([KT, 4 * T], BF16, tag="agst")    # o|o|h|h
            ssq_part = res.tile([KT, NT16], F32, tag="ssqp")
            ssq_red = res.tile([KT, 1], F32, tag="ssqr")
            s_col = res.tile([KT, 1], F32, tag="scol")
            ones_r = res.tile([1, KT], F32, tag="onesr")          # bcast lhsT
            ones_c = res.tile([KT, 1], F32, tag="onesc")          # fold rhs
            d_bf = res.tile([1, T], BF16, tag="dbf")
            d_rec = res.tile([1, T], F32, tag="drec")
            sval = res.tile([1, 1], F32, tag="sval")
            srec = res.tile([1, 1], F32, tag="srec")
            recb_s = res.tile([KT, T], F32, tag="recb")

            # ---- prologue ----
            nc.gpsimd.memset(ones_r[:], 1.0)
            nc.gpsimd.memset(ones_c[:], 1.0)
            nc.gpsimd.memset(ht_s[:], 0.0)
            nc.gpsimd.memset(c_s[:], 0.0)
            for t in range(NT16):
                nc.sync.dma_start(xpt_s[:, t * SS:(t + 1) * SS],
                                  xpt_d[t * KT:(t + 1) * KT, :])
                nc.sync.dma_start(xt_s[:, t * T:(t + 1) * T],
                                  xt_d[t * KT:(t + 1) * KT, :])
            for t in range(2):
                nc.sync.dma_start(xp1_s[:, t * NMA:(t + 1) * NMA],
                                  xp1_d[t * KT:(t + 1) * KT, :])
            for t in range(NKW):
                nc.sync.dma_start(wt_s[:, t * G:(t + 1) * G],
                                  wt_d[t * KT:(t + 1) * KT, :])

            for step in range(STEPS):
                first = step == 0
                last = step == STEPS - 1
                if trace_steps:
                    nc.scalar.print(f"step {step}")
                src_s = xt_s if first else xq_s

                if not first:
                    # consume previous AllGather: xq = x + q (in place), reload h
                    for t in range(NT16):
                        j, half = t // 2, t % 2
                        qrow = j * (2 * FS) + half * KT
                        xqt = xq_s[:, t * T:(t + 1) * T]
                        nc.sync.dma_start(xqt, ag_out[qrow:qrow + KT, :])
                        nc.vector.tensor_tensor(
                            xqt, xqt, xt_s[:, t * T:(t + 1) * T], OP.add)
                        nc.sync.dma_start(
                            ht_s[:, t * T:(t + 1) * T],
                            ag_out[qrow + FS:qrow + FS + KT, :])
                    # sum(xq^2) -> scale s = 1/sqrt(ssq*sxp), broadcast [128,1]
                    for t in range(NT16):
                        xqt = xq_s[:, t * T:(t + 1) * T]
                        nc.vector.scalar_tensor_tensor(
                            tcand_bf[:, 0:T], xqt, 1.0, xqt, OP.mult, OP.mult,
                            accum_out=ssq_part[:, t:t + 1])
                    nc.vector.reduce_sum(ssq_red[:], ssq_part[:], AX)
                    ps1 = psum.tile([KT, T], F32, tag="ps")
                    nc.tensor.matmul(ps1[:1, :1], ssq_red[:], ones_c[:],
                                     start=True, stop=True)
                    nc.scalar.activation(sval[:], ps1[:1, :1], AF.Sqrt,
                                         scale=float(sxp))
                    nc.vector.reciprocal(srec[:], sval[:])
                    ps2 = psum.tile([KT, T], F32, tag="ps")
                    nc.tensor.matmul(ps2[:, :1], ones_r[:], srec[:],
                                     start=True, stop=True)
                    nc.scalar.activation(s_col[:], ps2[:, :1], AF.Copy)

                # ---- attention: eT = xp_k @ xq^T  [256, 512] ----
                for mi in range(2):
                    ep = psum.tile([KT, T], F32, tag="ps")
                    for t in range(NT16):
                        nc.tensor.matmul(
                            ep[:],
                            xpt_s[:, t * SS + mi * KT: t * SS + (mi + 1) * KT],
                            src_s[:, t * T:(t + 1) * T],
                            start=(t == 0), stop=(t == NT16 - 1))
                    nc.scalar.activation(
                        expe_s[:, mi * T:(mi + 1) * T], ep[:], AF.Exp,
                        scale=(float(s1_scale) if first else s_col[:]))

                # ---- numerator: NT = [xp_k | 1]^T @ expe  [2049, 512] ----
                nt_in = dram.tile([NMA, T], BF16, tag="ntin")
                nt_out = dram.tile([NMA, T], BF16, tag="ntout",
                                   addr_space="Shared")
                for mi in range(NMA // KT + 1):
                    mw = min(KT, NMA - mi * KT)
                    nps = psum.tile([KT, T], F32, tag="ps")
                    for t in range(2):
                        nc.tensor.matmul(
                            nps[:mw, :],
                            xp1_s[:, t * NMA + mi * KT: t * NMA + mi * KT + mw],
                            expe_s[:, t * T:(t + 1) * T],
                            start=(t == 0), stop=(t == 1))
                    nt_st = stage.tile([KT, T], BF16, tag="ntst", bufs=2)
                    nc.any.tensor_copy(nt_st[:mw, :], nps[:mw, :])
                    nc.sync.dma_start(nt_in[mi * KT: mi * KT + mw, :],
                                      nt_st[:mw, :])
                nc.gpsimd.collective_compute(
                    "AllReduce", OP.add, replica_groups=rg,
                    ins=[nt_in.opt()], outs=[nt_out.opt()])

                # ---- r = N / d ----
                nc.sync.dma_start(d_bf[:], nt_out[S:S + 1, :])
                nc.vector.reciprocal(d_rec[:], d_bf[:])
                ps3 = psum.tile([KT, T], F32, tag="ps")
                nc.tensor.matmul(ps3[:], ones_r[:], d_rec[:],
                                 start=True, stop=True)
                nc.scalar.activation(recb_s[:], ps3[:], AF.Copy)
                for t in range(NT16):
                    rtt = rt_s[:, t * T:(t + 1) * T]
                    nc.sync.dma_start(rtt, nt_out[t * KT:(t + 1) * KT, :])
                    nc.vector.tensor_tensor(rtt, rtt, recb_s[:], OP.mult)

                # ---- z^T = W~^T @ [xq; r; h]  (+ C), gates ----
                m_list = [6, 7] if last else list(range(NM))
                for m in m_list:
                    zp = psum.tile([KT, T], F32, tag="ps")
                    kk = 0
                    for seg_s in (src_s, rt_s, ht_s):
                        for t in range(NT16):
                            nc.tensor.matmul(
                                zp[:],
                                wt_s[:, kk * G + m * KT: kk * G + (m + 1) * KT],
                                seg_s[:, t * T:(t + 1) * T],
                                start=(kk == 0), stop=(kk == NKW - 1))
                            kk += 1
                    gate, half = m // 2, m % 2
                    hs = slice(half * T, (half + 1) * T)
                    ct_st = stage.tile([KT, T], F32, tag="ctst", bufs=2)
                    nc.sync.dma_start(ct_st[:], ctp_d[m * KT:(m + 1) * KT, :])
                    if gate == 2:  # candidate: tanh(z + C)
                        nc.vector.tensor_tensor(tmp_f[:], zp[:], ct_st[:],
                                                OP.add)
                        nc.scalar.activation(tcand_bf[:, hs], tmp_f[:],
                                             AF.Tanh)
                    else:  # hard sigmoid: clip(0.2*z + (0.2*C+0.5), 0, 1)
                        nc.vector.scalar_tensor_tensor(
                            tmp_f[:], zp[:], 0.2, ct_st[:], OP.mult, OP.add)
                        if gate == 0:
                            dst = i_bf[:, hs]
                        elif gate == 1:
                            dst = f_bf[:, hs]
                        elif last:
                            dst = tmp_f[:]  # o in f32, clipped in place
                        else:
                            dst = ag_stage[:, hs]  # o slot (bf16)
                        nc.vector.tensor_scalar(dst, tmp_f[:], 0.0, 1.0,
                                                OP.max, OP.min)
                        if gate == 3 and last:
                            nc.sync.dma_start(
                                o_out_d[half * KT:(half + 1) * KT, :], dst)

                if last:
                    continue

                # ---- c, h update ----
                for half in range(2):
                    hs = slice(half * T, (half + 1) * T)
                    nc.vector.tensor_tensor(c_s[:, hs], c_s[:, hs],
                                            f_bf[:, hs], OP.mult)
                    nc.vector.tensor_tensor(tmp_f[:], i_bf[:, hs],
                                            tcand_bf[:, hs], OP.mult)
                    nc.vector.tensor_tensor(c_s[:, hs], c_s[:, hs],
                                            tmp_f[:], OP.add)
                    nc.scalar.activation(tanhc_bf[:, hs], c_s[:, hs], AF.Tanh)
                    nc.vector.tensor_tensor(
                        ag_stage[:, (2 + half) * T:(3 + half) * T],
                        ag_stage[:, hs], tanhc_bf[:, hs], OP.mult)

                # ---- AllGather of (o, h) slices ----
                ag_in = dram.tile([2 * FS, T], BF16, tag="agin")
                ag_out = dram.tile([NCORES * 2 * FS, T], BF16, tag="agout",
                                   addr_space="Shared")
                for t in range(4):
                    nc.sync.dma_start(ag_in[t * KT:(t + 1) * KT, :],
                                      ag_stage[:, t * T:(t + 1) * T])
                nc.gpsimd.collective_compute(
                    "AllGather", OP.bypass, replica_groups=rg,
                    ins=[ag_in.opt()], outs=[ag_out.opt()])

    nc.compile()
    return nc


def _prep_inputs(x, xp, W, U, b):
    """Host-side sharding, packing, and bf16 conversion."""
    x64 = x.astype(np.float64)
    sxp = float(np.sum(xp.astype(np.float64) ** 2))
    ssx = float(np.sum(x64 ** 2))
    s1 = 1.0 / (np.sqrt(ssx * sxp) + 1e-7)

    xt_bf = np.ascontiguousarray(x.T).astype(nbf)
    in_maps = []
    for k in range(NCORES):
        cols = np.concatenate(
            [np.arange(g * F + k * FS, g * F + (k + 1) * FS) for g in range(4)])
        wt = np.concatenate([W[:F, cols], W[F:, cols], U[:, cols]],
                            axis=0).astype(nbf)
        ct = (b[cols][None, :].astype(np.float64)
              - x64 @ W[:F, cols].astype(np.float64)).T  # [1024, 512]
        ctp = ct.copy()
        ctp[:2 * FS] = 0.2 * ct[:2 * FS] + 0.5       # i, f
        ctp[3 * FS:] = 0.2 * ct[3 * FS:] + 0.5       # o
        xpk = xp[k * SS:(k + 1) * SS, :]
        xpt = np.ascontiguousarray(xpk.T).astype(nbf)
        xp1 = np.concatenate(
            [xpk, np.ones((SS, 1), np.float32)], axis=1).astype(nbf)
        in_maps.append({
            "wt": np.ascontiguousarray(wt),
            "ctp": np.ascontiguousarray(ctp.astype(np.float32)),
            "xpt": xpt,
            "xp1": np.ascontiguousarray(xp1),
            "xt": xt_bf,
        })
    return in_maps, s1, sxp


def kernel(x, xp, q_init, W, U, b, _trace=False, _tmpdir=None):
    x = np.asarray(x, np.float32)
    xp = np.asarray(xp, np.float32)
    W = np.asarray(W, np.float32)
    U = np.asarray(U, np.float32)
    b = np.asarray(b, np.float32)

    in_maps, s1, sxp = _prep_inputs(x, xp, W, U, b)
    nc = _build(s1, sxp)
    res = run_bass_kernel_spmd(nc, in_maps, core_ids=list(range(NCORES)),
                               trace=_trace, tmpdir=_tmpdir)
    qT = np.concatenate([res.results[k]["o_out"] for k in range(NCORES)],
                        axis=0)  # [2048, 512]
    out0 = x + qT.T.astype(np.float32)
    if _trace:
        kernel.last_result = res
    return (out0, xp)
